# revision 14
# baseline (speedup 1.0000x reference)
"""Multi-head attention (B=4, S=2048, D=512, H=8, E=64) on 8 TRN2 NeuronCores.

Sharding: core c -> batch c//2, query rows [(c%2)*1024, (c%2)*1024+1024).
Each core holds full K/V of its batch and computes all 8 heads for its
query half end-to-end (projections, softmax attention, out projection);
the host only slices/casts inputs and concatenates the per-core outputs.

Per-core layout (the PE contracts over partitions, so data is kept
transposed with the contraction axis on partitions):
  - q/k/v arrive as bf16 (host cast) and are transposed to [d, s] layout
    by the DMA xbar directly from DRAM.
  - per-head projections are packed in pairs (2 heads x E=64 -> M=128),
    biases added on the DVE during PSUM evacuation (per-partition scalar).
  - S^T[t,q] = khT.T @ qhT per head (K=64); exp runs on ScalarE with the
    1/sqrt(E) scale folded in; no max-subtraction (scores are O(1)).
  - PV uses an augmented V projection vh_aug that carries an all-ones
    column per head, so OT = vh_aug.T @ expS^T yields the attention
    numerator AND the softmax denominators in the same matmul chain.
    Every head uses the block [E(64) | ones] -> M=65 at base partition 0
    (attention output on partitions 0..63, sums on 64); concat^T is stored
    per-head [64, H, sq] and the out projection contracts per head (K=64),
    since matmul quadrant placements other than (64,0)/(0,0) are rejected
    by codegen and DVE/ACT lanes cannot shift partitions.
  - normalization: reciprocal of the sums row, broadcast across the 64
    output partitions with a K=1 matmul, then multiplied into O^T during
    the PSUM->SBUF evacuation that also builds concat^T.
  - out projection: Y[q,:] = concat^T.T @ Wo^T with K=128 pair tiles.
"""

import numpy as np
import ml_dtypes

import concourse.bacc as bacc
import concourse.mybir as mybir
import concourse.tile as tile
from concourse import bass_utils

P = 128
D = 512
H = 8
E = 64
NG = H // 2
B_FULL, S_FULL = 4, 2048
N_CORES = 8
SQ = 1024              # per-core query rows
SK = 2048              # per-core key rows

# augmented vh column blocks: 65 wide per head ([E(64) | ones])
WIDTHS = [65 for h in range(H)]
OFF = np.cumsum([0] + WIDTHS).tolist()
A = OFF[-1]            # 644

F32 = mybir.dt.float32
F32R = mybir.dt.float32r
BF16 = mybir.dt.bfloat16

# dt_in: transposed inputs + projection weights; dt_att: qhT/khT;
# dt_p: expS/vh_aug; dt_out: concatT/WoT
DEFAULT_CFG = {"dt_in": BF16, "dt_att": F32R, "dt_p": BF16, "dt_out": F32R}


def build_nc(sq=SQ, sk=SK, cfg=None, repeat=1):
    cfg = dict(DEFAULT_CFG, **(cfg or {}))
    dt_in, dt_att, dt_p, dt_out = (
        cfg["dt_in"], cfg["dt_att"], cfg["dt_p"], cfg["dt_out"])
    sqt, skt, ndt = sq // P, sk // P, D // P
    qcs = min(512, sq)
    nqc = sq // qcs
    kcs = min(512, sk)
    nkc = sk // kcs
    tchunk = min(512, sk)  # rows per DMA-transpose call

    nc = bacc.Bacc("TRN2", target_bir_lowering=False, debug=False)
    di = {}
    for name, shape, dt in [
        ("q_loc", [sq, D], dt_in), ("k_loc", [sk, D], dt_in), ("v_loc", [sk, D], dt_in),
        ("Wqg", [NG, D, P], dt_in), ("Wkg", [NG, D, P], dt_in),
        ("bqg", [P, NG], F32), ("bkg", [P, NG], F32),
        ("Wv_aug", [D + 1, A], dt_in), ("WoTh", [64, H, D], F32),
    ]:
        di[name] = nc.dram_tensor(name, shape, dt, kind="ExternalInput").ap()
    y_t = nc.dram_tensor("y_loc", [sq, D], F32, kind="ExternalOutput").ap()

    from contextlib import ExitStack
    with tile.TileContext(nc) as tc, ExitStack() as top:
        pers = top.enter_context(tc.tile_pool(name="pers", bufs=1))
        wq = pers.tile([P, NG, ndt, P], dt_in, name="wq")
        wk = pers.tile([P, NG, ndt, P], dt_in, name="wk")
        wv = pers.tile([P, ndt, A], dt_in, name="wv")
        wv1 = pers.tile([1, A], dt_in, name="wv1")
        wo = pers.tile([64, H, D], dt_out, name="wo")
        bq_sb = pers.tile([P, NG], F32, name="bq_sb")
        bk_sb = pers.tile([P, NG], F32, name="bk_sb")
        ones128 = pers.tile([1, P], dt_in, name="ones128")
        sel0 = pers.tile([P, 64], F32R, name="sel0")
        sel0f = pers.tile([P, 64], F32, name="sel0f")
        qh = pers.tile([P, NG, sq], dt_att, name="qh")
        kh = pers.tile([P, NG, sk], dt_att, name="kh")
        vh = pers.tile([P, skt, A], dt_p, name="vh")
        rc_sb = pers.tile([P, sq], F32R, name="rc_sb")
        rs_sb = pers.tile([64, sq], F32, name="rs_sb")
        cT = pers.tile([64, H, sq], dt_out, name="cT")

        def body():
            with ExitStack() as es:
                xT = es.enter_context(tc.tile_pool(name="xT", bufs=1))
                ps = es.enter_context(tc.tile_pool(name="ps", bufs=1, space="PSUM"))
                sb = es.enter_context(tc.tile_pool(name="sbw", bufs=1))

                nc.vector.memset(ones128[:], 1.0)
                nc.vector.memset(sel0f[:], 0.0)
                nc.vector.memset(sel0f[0:1, :], 1.0)
                nc.vector.memset(sel0f[64:65, :], 1.0)
                nc.vector.tensor_copy(sel0[:], sel0f[:])
                zf = sb.tile([P, sq], F32, tag="zf", name="zf", bufs=1)
                nc.vector.memset(zf[:], 0.0)
                nc.vector.tensor_copy(rc_sb[:], zf[:])

                # ---- weight loads ----
                nc.sync.dma_start(wq[:], di["Wqg"].rearrange("g (do di) m -> di g do m", di=P))
                nc.sync.dma_start(wk[:], di["Wkg"].rearrange("g (do di) m -> di g do m", di=P))
                nc.sync.dma_start(wv[:], di["Wv_aug"][0:D].rearrange("(do di) m -> di do m", di=P))
                nc.sync.dma_start(wv1[:], di["Wv_aug"][D:D + 1, :])
                nc.gpsimd.dma_start(wo[:], di["WoTh"])
                nc.sync.dma_start(bq_sb[:], di["bqg"])
                nc.sync.dma_start(bk_sb[:], di["bkg"])

                # ---- transposes [s,d] -> [d,s] straight from DRAM via xbar ----
                qT = xT.tile([P, ndt, sq], dt_in, name="qT")
                kT = xT.tile([P, ndt, sk], dt_in, name="kT")
                vT = xT.tile([P, ndt, sk], dt_in, name="vT")
                for o in range(sq // tchunk):
                    nc.sync.dma_start_transpose(
                        qT[:, :, o * tchunk:(o + 1) * tchunk],
                        di["q_loc"][o * tchunk:(o + 1) * tchunk, :])
                for o in range(sk // tchunk):
                    nc.sync.dma_start_transpose(
                        kT[:, :, o * tchunk:(o + 1) * tchunk],
                        di["k_loc"][o * tchunk:(o + 1) * tchunk, :])
                    nc.sync.dma_start_transpose(
                        vT[:, :, o * tchunk:(o + 1) * tchunk],
                        di["v_loc"][o * tchunk:(o + 1) * tchunk, :])

                # ---- projections (head pairs packed to M=128) ----
                for g in range(NG):
                    for c in range(nqc):
                        pq = ps.tile([P, 1024], F32, tag="big", name=f"pq_{g}_{c}", bufs=2)
                        for t in range(ndt):
                            nc.tensor.matmul(
                                pq[:, :qcs], wq[:, g, t, :],
                                qT[:, t, c * qcs:(c + 1) * qcs],
                                start=(t == 0), stop=(t == ndt - 1))
                        nc.vector.tensor_scalar_add(
                            qh[:, g, c * qcs:(c + 1) * qcs], pq[:, :qcs], bq_sb[:, g:g + 1])
                    for c in range(nkc):
                        pk = ps.tile([P, 1024], F32, tag="big", name=f"pk_{g}_{c}", bufs=2)
                        for t in range(ndt):
                            nc.tensor.matmul(
                                pk[:, :kcs], wk[:, g, t, :],
                                kT[:, t, c * kcs:(c + 1) * kcs],
                                start=(t == 0), stop=(t == ndt - 1))
                        nc.vector.tensor_scalar_add(
                            kh[:, g, c * kcs:(c + 1) * kcs], pk[:, :kcs], bk_sb[:, g:g + 1])
                # augmented V projection (bias + ones via a K=1 matmul)
                for tt in range(skt):
                    pv = ps.tile([P, 1024], F32, tag="big", name=f"pv_{tt}", bufs=2)
                    for t in range(ndt):
                        nc.tensor.matmul(pv[:, 0:512], vT[:, t, tt * P:(tt + 1) * P],
                                         wv[:, t, 0:512], start=(t == 0), stop=False)
                        nc.tensor.matmul(pv[:, 512:A], vT[:, t, tt * P:(tt + 1) * P],
                                         wv[:, t, 512:A], start=(t == 0), stop=False)
                    nc.tensor.matmul(pv[:, 0:512], ones128[:],
                                     wv1[:, 0:512], start=False, stop=True)
                    nc.tensor.matmul(pv[:, 512:A], ones128[:],
                                     wv1[:, 512:A], start=False, stop=True)
                    nc.vector.tensor_copy(vh[:, tt, :], pv[:, 0:A])

                # ---- attention per head ----
                for h in range(H):
                    g, po = h // 2, (h % 2) * 64
                    ot = ps.tile([P, sq], F32, tag="ot", name=f"ot_{h}", bufs=2)
                    for tt in range(skt):
                        st = ps.tile([P, sq], F32, tag="big", name=f"st_{h}_{tt}", bufs=2)
                        for c in range(nqc):
                            nc.tensor.matmul(
                                st[:, c * qcs:(c + 1) * qcs],
                                kh[po:po + 64, g, tt * P:(tt + 1) * P],
                                qh[po:po + 64, g, c * qcs:(c + 1) * qcs],
                                start=True, stop=True)
                        ex = sb.tile([P, sq], dt_p, tag="ex", name=f"ex_{h}_{tt}", bufs=6)
                        nc.scalar.activation(ex[:], st[:],
                                             mybir.ActivationFunctionType.Exp, scale=0.125)
                        for c in range(nqc):
                            nc.tensor.matmul(
                                ot[0:65, c * qcs:(c + 1) * qcs],
                                vh[:, tt, OFF[h]:OFF[h] + 65],
                                ex[:, c * qcs:(c + 1) * qcs],
                                start=(tt == 0), stop=(tt == skt - 1))
                    with nc.allow_low_precision("softmax denominator rounded to f32r"):
                        nc.vector.reciprocal(rc_sb[64:65, :], ot[64:65, :])
                    rp = ps.tile([P, sq], F32, tag="big", name=f"rp_{h}", bufs=2)
                    for c in range(nqc):
                        nc.tensor.matmul(rp[0:64, c * qcs:(c + 1) * qcs],
                                         sel0[64:128, :],
                                         rc_sb[64:128, c * qcs:(c + 1) * qcs],
                                         start=True, stop=True)
                    nc.vector.tensor_copy(rs_sb[:], rp[0:64, :])
                    nc.vector.tensor_tensor(cT[:, h, :], ot[0:64, :],
                                            rs_sb[:], mybir.AluOpType.mult)

                # ---- output projection ----
                for qt in range(sqt):
                    yp = ps.tile([P, 1024], F32, tag="big", name=f"yp_{qt}", bufs=2)
                    for h in range(H):
                        nc.tensor.matmul(yp[:, 0:512], cT[:, h, qt * P:(qt + 1) * P],
                                         wo[:, h, :], start=(h == 0), stop=(h == H - 1))
                    ys = sb.tile([P, 512], F32, tag="y", name=f"ys_{qt}", bufs=3)
                    nc.vector.tensor_copy(ys[:], yp[:, 0:512])
                    nc.sync.dma_start(y_t[qt * P:(qt + 1) * P, :], ys[:])

        if repeat == 1:
            body()
        else:
            with tc.For_i(0, repeat, 1):
                body()

    nc.compile()
    return nc


def host_pack(Wq, bq, Wk, bk, Wv, bv, Wo):
    Wq, bq, Wk, bk, Wv, bv, Wo = [np.asarray(x, np.float32) for x in
                                  (Wq, bq, Wk, bk, Wv, bv, Wo)]
    bf = ml_dtypes.bfloat16
    Wqg = np.ascontiguousarray(np.stack(
        [np.concatenate([Wq[2 * g], Wq[2 * g + 1]], axis=1) for g in range(NG)])).astype(bf)
    Wkg = np.ascontiguousarray(np.stack(
        [np.concatenate([Wk[2 * g], Wk[2 * g + 1]], axis=1) for g in range(NG)])).astype(bf)
    bqg = np.ascontiguousarray(np.stack(
        [np.concatenate([bq[2 * g], bq[2 * g + 1]]) for g in range(NG)], axis=1))
    bkg = np.ascontiguousarray(np.stack(
        [np.concatenate([bk[2 * g], bk[2 * g + 1]]) for g in range(NG)], axis=1))
    Wv_aug = np.zeros((D + 1, A), np.float32)
    for h in range(H):
        o = OFF[h]
        Wv_aug[:D, o:o + 64] = Wv[h]
        Wv_aug[D, o:o + 64] = bv[h]
        Wv_aug[D, o + 64] = 1.0
    WoTh = np.ascontiguousarray(Wo.T.reshape(H, 64, D).transpose(1, 0, 2))
    return {"Wqg": Wqg, "Wkg": Wkg, "bqg": bqg, "bkg": bkg,
            "Wv_aug": Wv_aug.astype(bf), "WoTh": WoTh}


def make_core_input(q_loc, k_loc, v_loc, packed):
    bf = ml_dtypes.bfloat16
    return {
        "q_loc": np.ascontiguousarray(q_loc).astype(bf),
        "k_loc": np.ascontiguousarray(k_loc).astype(bf),
        "v_loc": np.ascontiguousarray(v_loc).astype(bf),
        **packed,
    }


_NC_CACHE = {}


def _get_nc(repeat=1):
    if repeat not in _NC_CACHE:
        _NC_CACHE[repeat] = build_nc(repeat=repeat)
    return _NC_CACHE[repeat]


def make_in_maps(q, k, v, Wq, bq, Wk, bk, Wv, bv, Wo):
    q, k, v = [np.asarray(x, np.float32) for x in (q, k, v)]
    packed = host_pack(Wq, bq, Wk, bk, Wv, bv, Wo)
    return [
        make_core_input(q[c // 2, (c % 2) * SQ:(c % 2) * SQ + SQ],
                        k[c // 2], v[c // 2], packed)
        for c in range(N_CORES)
    ]


def assemble(results):
    out = np.empty((B_FULL, S_FULL, D), np.float32)
    for c in range(N_CORES):
        b, qlo = c // 2, (c % 2) * SQ
        out[b, qlo:qlo + SQ] = results[c]["y_loc"]
    return out


def kernel(q, k, v, Wq, bq, Wk, bk, Wv, bv, Wo):
    nc = _get_nc(repeat=1)
    in_maps = make_in_maps(q, k, v, Wq, bq, Wk, bk, Wv, bv, Wo)
    res = bass_utils.run_bass_kernel_spmd(nc, in_maps, core_ids=list(range(N_CORES)))
    return assemble(res.results)


# revision 22
# speedup vs baseline: 2.7536x; 2.7536x over previous
"""Multi-head attention (B=4, S=2048, D=512, H=8, E=64) on 8 TRN2 NeuronCores.

Sharding: core c -> batch c//2, query rows [(c%2)*1024, (c%2)*1024+1024).
Each core holds full K/V of its batch and computes all 8 heads for its
query half end-to-end (projections, softmax attention, out projection);
the host only slices/casts inputs and concatenates the per-core outputs.

Per-core layout (the PE contracts over partitions, so data is kept
transposed with the contraction axis on partitions):
  - q/k/v arrive as bf16 (host cast) and are transposed to [d, s] layout
    by the DMA xbar directly from DRAM.
  - per-head projections are packed in pairs (2 heads x E=64 -> M=128),
    biases added on the DVE during PSUM evacuation (per-partition scalar).
  - qh/kh are stored per head zero-padded to K=128 (even heads occupy
    partitions 0..63, odd heads 64..127, the other half zeroed), so the
    S^T matmuls run as full 128x128-mode matmuls.  K=64 operands would be
    emitted as 2x-row-tiled quadrant matmuls, and every S <-> PV
    alternation would then switch the PE tiling mode, which drains the
    array each time (~256 drains per pass).
  - S^T[t,q] = khp_h.T @ qhp_h; exp on ScalarE with the 1/sqrt(E) scale
    folded in; no max-subtraction (scores are O(1) by construction).
  - PV uses an augmented V projection vh_aug: per head [E(64) | ones], so
    OT = vh_aug.T @ expS^T gives the attention numerator (partitions
    0..63) and the softmax denominators (partition 64) in one chain.
  - normalization: reciprocal of the sums row, broadcast across the 64
    output partitions with a selector matmul (constant K=64 stationary
    with a single ones row -- keeps legal quadrant placement), multiplied
    into O^T during the PSUM->SBUF evacuation that builds concat^T.
  - out projection contracts per head (K=64): Y[q,:] += cT_h.T @ WoTh_h.
"""

import numpy as np
import ml_dtypes

import concourse.bacc as bacc
import concourse.mybir as mybir
import concourse.tile as tile
from concourse import bass_utils

P = 128
D = 512
H = 8
E = 64
NG = H // 2
B_FULL, S_FULL = 4, 2048
N_CORES = 8
SQ = 1024              # per-core query rows
SK = 2048              # per-core key rows

# augmented vh column blocks: 65 wide per head ([E(64) | ones])
WIDTHS = [65 for h in range(H)]
OFF = np.cumsum([0] + WIDTHS).tolist()
A = OFF[-1]            # 520

F32 = mybir.dt.float32
F32R = mybir.dt.float32r
BF16 = mybir.dt.bfloat16

# dt_in: transposed inputs + projection weights; dt_att: qhp/khp;
# dt_p: expS/vh_aug; dt_out: concatT/WoTh
DEFAULT_CFG = {"dt_in": BF16, "dt_att": BF16, "dt_p": BF16, "dt_out": F32R}


def build_nc(sq=SQ, sk=SK, cfg=None, repeat=1, phases=4,
             bigbufs=3, otbufs=1, exbufs=8, att_mode="full", st_bf16=True):
    cfg = dict(DEFAULT_CFG, **(cfg or {}))
    dt_in, dt_att, dt_p, dt_out = (
        cfg["dt_in"], cfg["dt_att"], cfg["dt_p"], cfg["dt_out"])
    sqt, skt, ndt = sq // P, sk // P, D // P
    qcs = min(512, sq)
    nqc = sq // qcs
    kcs = min(512, sk)
    nkc = sk // kcs
    tchunk = min(512, sk)  # rows per DMA-transpose call

    nc = bacc.Bacc("TRN2", target_bir_lowering=False, debug=False)
    di = {}
    for name, shape, dt in [
        ("q_loc", [sq, D], dt_in), ("k_loc", [sk, D], dt_in), ("v_loc", [sk, D], dt_in),
        ("Wqg", [NG, D, P], dt_in), ("Wkg", [NG, D, P], dt_in),
        ("bqg", [P, NG], F32), ("bkg", [P, NG], F32),
        ("Wv_aug", [D + 1, A], dt_in), ("WoTh", [64, H, D], F32),
    ]:
        di[name] = nc.dram_tensor(name, shape, dt, kind="ExternalInput").ap()
    y_t = nc.dram_tensor("y_loc", [sq, D], F32, kind="ExternalOutput").ap()

    from contextlib import ExitStack
    with tile.TileContext(nc) as tc, ExitStack() as top:
        pers = top.enter_context(tc.tile_pool(name="pers", bufs=1))
        wq = pers.tile([P, NG, ndt, P], dt_in, name="wq")
        wk = pers.tile([P, NG, ndt, P], dt_in, name="wk")
        wv = pers.tile([P, ndt, A], dt_in, name="wv")
        wv1 = pers.tile([1, A], dt_in, name="wv1")
        wo = pers.tile([64, H, D], dt_out, name="wo")
        bq_sb = pers.tile([P, NG], F32, name="bq_sb")
        bk_sb = pers.tile([P, NG], F32, name="bk_sb")
        ones128 = pers.tile([1, P], dt_in, name="ones128")
        sel0 = pers.tile([P, 64], F32R, name="sel0")
        sel0f = pers.tile([P, 64], F32, name="sel0f")
        qhp = pers.tile([P, H, sq], dt_att, name="qhp")
        khp = pers.tile([P, H, sk], dt_att, name="khp")
        vh = pers.tile([P, skt, A], dt_p, name="vh")
        rc_sb = pers.tile([P, sq], F32R, name="rc_sb")
        rs_sb = pers.tile([64, sq], F32, name="rs_sb")
        cT = pers.tile([64, H, sq], dt_out, name="cT")

        def body():
            with ExitStack() as es:
                xT = es.enter_context(tc.tile_pool(name="xT", bufs=1))
                ps = es.enter_context(tc.tile_pool(name="ps", bufs=1, space="PSUM"))
                sb = es.enter_context(tc.tile_pool(name="sbw", bufs=1))

                nc.vector.memset(ones128[:], 1.0)
                nc.vector.memset(sel0f[:], 0.0)
                nc.vector.memset(sel0f[64:65, :], 1.0)
                nc.vector.tensor_copy(sel0[:], sel0f[:])
                zf = sb.tile([P, sq], F32, tag="zf", name="zf", bufs=1)
                nc.vector.memset(zf[:], 0.0)
                nc.vector.tensor_copy(rc_sb[:], zf[:])
                # zero the padded halves of qhp/khp (once per pass)
                nc.vector.memset(qhp[:], 0.0)
                nc.vector.memset(khp[:], 0.0)

                # ---- weight loads ----
                nc.sync.dma_start(wq[:], di["Wqg"].rearrange("g (do di) m -> di g do m", di=P))
                nc.sync.dma_start(wk[:], di["Wkg"].rearrange("g (do di) m -> di g do m", di=P))
                nc.sync.dma_start(wv[:], di["Wv_aug"][0:D].rearrange("(do di) m -> di do m", di=P))
                nc.sync.dma_start(wv1[:], di["Wv_aug"][D:D + 1, :])
                nc.gpsimd.dma_start(wo[:], di["WoTh"])
                nc.sync.dma_start(bq_sb[:], di["bqg"])
                nc.sync.dma_start(bk_sb[:], di["bkg"])

                # ---- transposes [s,d] -> [d,s] straight from DRAM via xbar ----
                qT = xT.tile([P, ndt, sq], dt_in, name="qT")
                kT = xT.tile([P, ndt, sk], dt_in, name="kT")
                vT = xT.tile([P, ndt, sk], dt_in, name="vT")
                for o in range(sq // tchunk):
                    nc.sync.dma_start_transpose(
                        qT[:, :, o * tchunk:(o + 1) * tchunk],
                        di["q_loc"][o * tchunk:(o + 1) * tchunk, :])
                for o in range(sk // tchunk):
                    nc.sync.dma_start_transpose(
                        kT[:, :, o * tchunk:(o + 1) * tchunk],
                        di["k_loc"][o * tchunk:(o + 1) * tchunk, :])
                    nc.sync.dma_start_transpose(
                        vT[:, :, o * tchunk:(o + 1) * tchunk],
                        di["v_loc"][o * tchunk:(o + 1) * tchunk, :])

                if phases < 2:
                    return
                # ---- projections (head pairs packed to M=128) ----
                for g in range(NG):
                    for c in range(nqc):
                        pq = ps.tile([P, 1024], F32, tag="st", name=f"pq_{g}_{c}", bufs=bigbufs)
                        for t in range(ndt):
                            nc.tensor.matmul(
                                pq[:, :qcs], wq[:, g, t, :],
                                qT[:, t, c * qcs:(c + 1) * qcs],
                                start=(t == 0), stop=(t == ndt - 1))
                        sl = slice(c * qcs, (c + 1) * qcs)
                        nc.vector.tensor_scalar_add(
                            qhp[0:64, 2 * g, sl], pq[0:64, :qcs], bq_sb[0:64, g:g + 1])
                        nc.vector.tensor_scalar_add(
                            qhp[64:128, 2 * g + 1, sl], pq[64:128, :qcs],
                            bq_sb[64:128, g:g + 1])
                    for c in range(nkc):
                        pk = ps.tile([P, 1024], F32, tag="st", name=f"pk_{g}_{c}", bufs=bigbufs)
                        for t in range(ndt):
                            nc.tensor.matmul(
                                pk[:, :kcs], wk[:, g, t, :],
                                kT[:, t, c * kcs:(c + 1) * kcs],
                                start=(t == 0), stop=(t == ndt - 1))
                        sl = slice(c * kcs, (c + 1) * kcs)
                        nc.vector.tensor_scalar_add(
                            khp[0:64, 2 * g, sl], pk[0:64, :kcs], bk_sb[0:64, g:g + 1])
                        nc.vector.tensor_scalar_add(
                            khp[64:128, 2 * g + 1, sl], pk[64:128, :kcs],
                            bk_sb[64:128, g:g + 1])
                # augmented V projection (bias + ones via a K=1 matmul)
                for tt in range(skt):
                    pv = ps.tile([P, 1024], F32, tag="st", name=f"pv_{tt}", bufs=bigbufs)
                    for t in range(ndt):
                        nc.tensor.matmul(pv[:, 0:512], vT[:, t, tt * P:(tt + 1) * P],
                                         wv[:, t, 0:512], start=(t == 0), stop=False)
                        nc.tensor.matmul(pv[:, 512:A], vT[:, t, tt * P:(tt + 1) * P],
                                         wv[:, t, 512:A], start=(t == 0), stop=False)
                    nc.tensor.matmul(pv[:, 0:512], ones128[:],
                                     wv1[:, 0:512], start=False, stop=True)
                    nc.tensor.matmul(pv[:, 512:A], ones128[:],
                                     wv1[:, 512:A], start=False, stop=True)
                    nc.vector.tensor_copy(vh[:, tt, :], pv[:, 0:A])

                if phases < 3:
                    return
                # ---- attention per head (all matmuls full 128x128 mode) ----
                for h in range(H):
                    ot = ps.tile([P, sq], F32, tag="ot", name=f"ot_{h}", bufs=otbufs)
                    for tt in range(skt):
                        st = ps.tile([P, sq], F32, tag="st", name=f"st_{h}_{tt}", bufs=bigbufs)
                        for c in range(nqc):
                            nc.tensor.matmul(
                                st[:, c * qcs:(c + 1) * qcs],
                                khp[:, h, tt * P:(tt + 1) * P],
                                qhp[:, h, c * qcs:(c + 1) * qcs],
                                start=True, stop=True)
                        ex = sb.tile([P, sq], dt_p, tag="ex", name=f"ex_{h}_{tt}", bufs=exbufs)
                        nc.scalar.activation(ex[:], st[:],
                                             mybir.ActivationFunctionType.Exp, scale=0.125)
                        if att_mode != "no_pv":
                            for c in range(nqc):
                                nc.tensor.matmul(
                                    ot[0:65, c * qcs:(c + 1) * qcs],
                                    vh[:, tt, OFF[h]:OFF[h] + 65],
                                    ex[:, c * qcs:(c + 1) * qcs],
                                    start=(tt == 0), stop=(tt == skt - 1))
                    if att_mode == "no_pv":
                        for c in range(nqc):
                            nc.tensor.matmul(
                                ot[0:65, c * qcs:(c + 1) * qcs],
                                vh[:, 0, OFF[h]:OFF[h] + 65],
                                ex[:, c * qcs:(c + 1) * qcs],
                                start=True, stop=True)
                    with nc.allow_low_precision("softmax denominator rounded to f32r"):
                        nc.vector.reciprocal(rc_sb[64:65, :], ot[64:65, :])
                    rp = ps.tile([P, sq], F32, tag="st", name=f"rp_{h}", bufs=bigbufs)
                    for c in range(nqc):
                        nc.tensor.matmul(rp[0:64, c * qcs:(c + 1) * qcs],
                                         sel0[64:128, :],
                                         rc_sb[64:128, c * qcs:(c + 1) * qcs],
                                         start=True, stop=True)
                    nc.vector.tensor_copy(rs_sb[:], rp[0:64, :])
                    nc.vector.tensor_tensor(cT[:, h, :], ot[0:64, :],
                                            rs_sb[:], mybir.AluOpType.mult)

                if phases < 4:
                    return
                # ---- output projection (per-head K=64) ----
                for qt in range(sqt):
                    yp = ps.tile([P, 1024], F32, tag="st", name=f"yp_{qt}", bufs=bigbufs)
                    for h in range(H):
                        nc.tensor.matmul(yp[:, 0:512], cT[:, h, qt * P:(qt + 1) * P],
                                         wo[:, h, :], start=(h == 0), stop=(h == H - 1))
                    ys = sb.tile([P, 512], F32, tag="y", name=f"ys_{qt}", bufs=3)
                    nc.vector.tensor_copy(ys[:], yp[:, 0:512])
                    nc.sync.dma_start(y_t[qt * P:(qt + 1) * P, :], ys[:])

        if repeat == 1:
            body()
        else:
            with tc.For_i(0, repeat, 1):
                body()

    nc.compile()
    return nc


def host_pack(Wq, bq, Wk, bk, Wv, bv, Wo):
    Wq, bq, Wk, bk, Wv, bv, Wo = [np.asarray(x, np.float32) for x in
                                  (Wq, bq, Wk, bk, Wv, bv, Wo)]
    bf = ml_dtypes.bfloat16
    Wqg = np.ascontiguousarray(np.stack(
        [np.concatenate([Wq[2 * g], Wq[2 * g + 1]], axis=1) for g in range(NG)])).astype(bf)
    Wkg = np.ascontiguousarray(np.stack(
        [np.concatenate([Wk[2 * g], Wk[2 * g + 1]], axis=1) for g in range(NG)])).astype(bf)
    bqg = np.ascontiguousarray(np.stack(
        [np.concatenate([bq[2 * g], bq[2 * g + 1]]) for g in range(NG)], axis=1))
    bkg = np.ascontiguousarray(np.stack(
        [np.concatenate([bk[2 * g], bk[2 * g + 1]]) for g in range(NG)], axis=1))
    Wv_aug = np.zeros((D + 1, A), np.float32)
    for h in range(H):
        o = OFF[h]
        Wv_aug[:D, o:o + 64] = Wv[h]
        Wv_aug[D, o:o + 64] = bv[h]
        Wv_aug[D, o + 64] = 1.0
    WoTh = np.ascontiguousarray(Wo.T.reshape(H, 64, D).transpose(1, 0, 2))
    return {"Wqg": Wqg, "Wkg": Wkg, "bqg": bqg, "bkg": bkg,
            "Wv_aug": Wv_aug.astype(bf), "WoTh": WoTh}


def make_core_input(q_loc, k_loc, v_loc, packed):
    bf = ml_dtypes.bfloat16
    return {
        "q_loc": np.ascontiguousarray(q_loc).astype(bf),
        "k_loc": np.ascontiguousarray(k_loc).astype(bf),
        "v_loc": np.ascontiguousarray(v_loc).astype(bf),
        **packed,
    }


_NC_CACHE = {}


def _get_nc(repeat=1):
    if repeat not in _NC_CACHE:
        _NC_CACHE[repeat] = build_nc(repeat=repeat)
    return _NC_CACHE[repeat]


def make_in_maps(q, k, v, Wq, bq, Wk, bk, Wv, bv, Wo):
    q, k, v = [np.asarray(x, np.float32) for x in (q, k, v)]
    packed = host_pack(Wq, bq, Wk, bk, Wv, bv, Wo)
    return [
        make_core_input(q[c // 2, (c % 2) * SQ:(c % 2) * SQ + SQ],
                        k[c // 2], v[c // 2], packed)
        for c in range(N_CORES)
    ]


def assemble(results):
    out = np.empty((B_FULL, S_FULL, D), np.float32)
    for c in range(N_CORES):
        b, qlo = c // 2, (c % 2) * SQ
        out[b, qlo:qlo + SQ] = results[c]["y_loc"]
    return out


def kernel(q, k, v, Wq, bq, Wk, bk, Wv, bv, Wo):
    nc = _get_nc(repeat=1)
    in_maps = make_in_maps(q, k, v, Wq, bq, Wk, bk, Wv, bv, Wo)
    res = bass_utils.run_bass_kernel_spmd(nc, in_maps, core_ids=list(range(N_CORES)))
    return assemble(res.results)
